# revision 2
# baseline (speedup 1.0000x reference)
"""
Trainium2 Bass kernel for nn_CausalSelfAttention_5214090298017.

Reference computes (B=2, T=2048, C=768, H=12, HD=64):
    q,k,v = split_heads(x @ W{q,k,v}.T + b)          # [B,H,T,HD]
    att   = softmax(mask(q @ k.T / sqrt(HD)))        # key-padding mask from attn_mask1
    y     = (att @ v).merge_heads() @ Wp.T + bp      # [B,T,C]

Sharding: 8 cores = 2 (batch) x 4 (head-groups of 3 heads).  Each core
computes a partial output  sum_{h in group} (att_h @ v_h) @ Wp_rows_h
([T, C]); the host sums the 4 group partials per batch (row-parallel Wp)
and concatenates over batch.

Device-side layout choices (per core):
  - Q^T, K^T stored [head_dim, T]; heads 0/1 packed into one [128, T]
    tensor (partitions 0-63 / 64-127) so their score matmuls land on
    disjoint PE row-groups and run concurrently.  Head 2's Q^T/K^T are
    stored TWICE (partitions 0-63 and 64-127), which costs nothing: the
    projection's m=64 block is emitted as two col-tiled matmuls writing
    both PSUM halves concurrently.  The duplicate lets the score matmuls
    for two CONSECUTIVE key tiles of head 2 run concurrently too (kt_a
    on rows 0-63, kt_b on rows 64-127), so a key-tile pair costs 3
    matmul slots for 3 heads instead of 4.
  - S^T = (K^T-tile).T @ Q^T gives score tiles [keys=128, q] with KEYS
    on partitions: the key-padding mask and the 1/sqrt(HD) scale are
    applied for free by the Exp activation (per-partition bias + scale).
    Key tiles are processed in pairs sharing one [128, 1024] PSUM tile;
    a pair that is fully mask-clean needs a single wide Exp.
  - V stored [T_k, 65] per head with a ones-column appended: the PV
    matmul accumulates [Y^T | softmax-denominator] in one pass.  The PV
    for group i is emitted between the h0/h1 and h2 score matmuls of
    group i+1 so the PE never waits on the exp stream.
  - Normalization (per-query 1/denom, on the free axis of Y^T) uses
    reciprocal_approx_fast + a K=1 matmul to broadcast across
    partitions; the h0/h1 broadcasts are col-tiled into one slot.
  - Key compaction: only unmasked keys (~50%) are shipped/computed; the
    padded tail is killed by the same exp-bias mask.
  - Input DMA is striped over the three DMA-capable queues (sync, ACT,
    gpsimd) so the ~6 MB input load lands in ~1/3 the serial time; the
    tail's output DMA is striped the same way.
Matmul operands are fp16 (full PE rate + fast weight load; fp32 PSUM
accumulation throughout).
"""

import itertools
import math
import os
import sys
from contextlib import ExitStack

import numpy as np

sys.path.insert(0, "/opt/trn_rl_repo")

import concourse.bass as bass  # noqa: E402,F401
import concourse.tile as tile  # noqa: E402
from concourse import bacc, mybir  # noqa: E402
from concourse import bass_utils  # noqa: E402

F32 = mybir.dt.float32
F16 = mybir.dt.float16
U16 = mybir.dt.uint16
ONE_F16 = 0x3C00  # 1.0 in fp16 bits (memset can't take fp16 directly)

B, T, C, H = 2, 2048, 768, 12
HD = C // H          # 64
GROUPS = 4           # head-groups (tensor parallel)
HPG = H // GROUPS    # 3 heads per group
J = HPG * HD         # 192 local channels
NCORES = 8
SCALE = 1.0 / math.sqrt(HD)
MASK_NEG = -30000.0  # exp(-30000 + small) == 0.0

COMPACT = os.environ.get("ATTN_NO_COMPACT", "") == ""
QCW = 512            # query chunk width for the attention phase


def _nchunks(n, cap=512, lo=256):
    """Split n (multiple of 128) into (start, width) chunks in [lo, cap]."""
    assert n % 128 == 0
    out, pos, rem = [], 0, n
    while rem > 0:
        w = min(cap, rem)
        if rem - w != 0 and rem - w < lo:
            w = max(lo, ((rem - lo) // 128) * 128)
        out.append((pos, w))
        pos += w
        rem -= w
    return out


def _pairing(kk):
    """Key tiles in pairs (last one single if kk is odd)."""
    gs = []
    i = 0
    while i + 1 < kk:
        gs.append((i, i + 1))
        i += 2
    if i < kk:
        gs.append((i,))
    return gs


def build_nc(tk, share_x, clean_kk=0, debug_taps=False):
    """Build the per-core Bass program.  tk = padded key count (mult of 128)."""
    kk = tk // 128

    nc = bacc.Bacc("TRN2", target_bir_lowering=False, debug=False)

    xt = nc.dram_tensor("xt", [769, T], F16, kind="ExternalInput").ap()
    xtkv = xt if share_x else nc.dram_tensor("xtkv", [769, tk], F16, kind="ExternalInput").ap()
    wqT = nc.dram_tensor("wqT", [768, J], F16, kind="ExternalInput").ap()
    wkT = nc.dram_tensor("wkT", [768, J], F16, kind="ExternalInput").ap()
    wvT = nc.dram_tensor("wvT", [769, J], F16, kind="ExternalInput").ap()
    bqv = nc.dram_tensor("bqv", [J], F32, kind="ExternalInput").ap()
    bkv = nc.dram_tensor("bkv", [J], F32, kind="ExternalInput").ap()
    mb = nc.dram_tensor("mb", [tk], F32, kind="ExternalInput").ap()
    wpT = nc.dram_tensor("wpT", [J, 768], F16, kind="ExternalInput").ap()
    bp4 = nc.dram_tensor("bp4", [768], F16, kind="ExternalInput").ap()
    out = nc.dram_tensor("o", [T, 768], F16, kind="ExternalOutput").ap()

    with tile.TileContext(nc) as tc, ExitStack() as ctx:
        const = ctx.enter_context(tc.tile_pool(name="const", bufs=1))
        ppool = ctx.enter_context(tc.tile_pool(name="psum", bufs=4, space="PSUM"))
        stpool = ctx.enter_context(tc.tile_pool(name="stbig", bufs=2, space="PSUM"))
        espool = ctx.enter_context(tc.tile_pool(name="es", bufs=6))
        opool = ctx.enter_context(tc.tile_pool(name="osb", bufs=4))
        mpool = ctx.enter_context(tc.tile_pool(name="misc", bufs=3))

        # ---------------- persistent SBUF tensors ----------------
        xt_s = const.tile([128, 6, T], F16, tag="xt")
        if share_x:
            xt1_s = const.tile([1, T], F16, tag="xt1")
            xkv_s, xkv1_s = xt_s, xt1_s
        else:
            xkv_s = const.tile([128, 6, tk], F16, tag="xkv")
            xkv1_s = const.tile([1, tk], F16, tag="xkv1")
        wq_s = const.tile([128, 6, J], F16, tag="wq")
        wk_s = const.tile([128, 6, J], F16, tag="wk")
        wv_s = const.tile([128, 6, J], F16, tag="wv")
        wv1_s = const.tile([1, J], F16, tag="wv1")
        bq_s = const.tile([128, 2], F32, tag="bq")
        bk_s = const.tile([128, 2], F32, tag="bk")
        mb_s = const.tile([128, kk], F32, tag="mb")
        wpT01_s = const.tile([128, 768], F16, tag="wp01")
        wph_s = {2: const.tile([65, 768], F16, tag="wp2", name="wp2")}
        ones_s = const.tile([65, 128], F16, tag="ones")
        # heads 0/1 packed on partitions 0-63 / 64-127; head 2 duplicated
        # on both partition halves (enables key-tile-pair concurrency)
        qt01_s = const.tile([128, T], F16, tag="qt01")
        qt2_s = const.tile([128, T], F16, tag="qt2")
        kt01_s = const.tile([128, tk], F16, tag="kt01")
        kt2_s = const.tile([128, tk], F16, tag="kt2")
        v_s = [const.tile([128, kk, 65], F16, tag=f"v{h}", name=f"v{h}") for h in range(3)]
        yn01_s = const.tile([128, T], F16, tag="yn01")
        yn2_s = const.tile([65, T], F16, tag="yn2")
        warm_s = const.tile([128, 512], F16, tag="warm")

        # ---------------- input DMAs striped over 3 queues ----------------
        dq = itertools.cycle([nc.sync, nc.scalar, nc.gpsimd])

        def dma(dst, src):
            next(dq).dma_start(dst, src)

        kchunks = _nchunks(tk)
        qchunks = _nchunks(T)
        for ci in range(6):
            dma(wk_s[:, ci, :], wkT[ci * 128:(ci + 1) * 128, :])
        n0, nw = kchunks[0]
        for ci in range(6):
            if not share_x:
                dma(xkv_s[:, ci, n0:n0 + nw], xtkv[ci * 128:(ci + 1) * 128, n0:n0 + nw])
        for ci in range(6):
            dma(wv_s[:, ci, :], wvT[ci * 128:(ci + 1) * 128, :])
        dma(wv1_s[:, :], wvT[768:769, :])
        dma(bq_s[:, 0:1], bqv[0:128][:, None])
        dma(bq_s[0:64, 1:2], bqv[128:192][:, None])
        dma(bq_s[64:128, 1:2], bqv[128:192][:, None])
        dma(bk_s[:, 0:1], bkv[0:128][:, None])
        dma(bk_s[0:64, 1:2], bkv[128:192][:, None])
        dma(bk_s[64:128, 1:2], bkv[128:192][:, None])
        dma(mb_s[:, :], mb.rearrange("(o p) -> p o", p=128))
        for (n0, nw) in kchunks[1:]:
            for ci in range(6):
                if not share_x:
                    dma(xkv_s[:, ci, n0:n0 + nw], xtkv[ci * 128:(ci + 1) * 128, n0:n0 + nw])
        if not share_x:
            dma(xkv1_s[:, :], xtkv[768:769, :])
        for ci in range(6):
            dma(wq_s[:, ci, :], wqT[ci * 128:(ci + 1) * 128, :])
        n0, nw = qchunks[0]
        for ci in range(6):
            dma(xt_s[:, ci, n0:n0 + nw], xt[ci * 128:(ci + 1) * 128, n0:n0 + nw])
        rest0 = qchunks[1][0]
        for ci in range(6):
            dma(xt_s[:, ci, rest0:T], xt[ci * 128:(ci + 1) * 128, rest0:T])
        if share_x:
            dma(xt1_s[:, :], xt[768:769, :])
        dma(wpT01_s[:, :], wpT[0:128, :])
        dma(wph_s[2][0:64, :], wpT[128:192, :])
        dma(wph_s[2][64:65, :], bp4[None, :])

        nc.vector.memset(warm_s[:, :].bitcast(U16), 0)
        nc.vector.memset(ones_s[:, :].bitcast(U16), ONE_F16)
        nc.vector.memset(yn2_s[64:65, :].bitcast(U16), ONE_F16)
        for h in range(3):
            nc.vector.memset(v_s[h][:, :, 64:65].bitcast(U16), ONE_F16)
        # pre-load the ACT exp table while the input DMA streams (the
        # first real exp would otherwise pay the ~1.3us table load)
        wact = mpool.tile([64, 8], F16, tag="wact", name="wact")
        nc.scalar.activation(out=wact[:, :], in_=warm_s[0:64, 0:8],
                             func=mybir.ActivationFunctionType.Exp,
                             bias=0.0, scale=1.0)

        # ---------------- projections (emitted just-in-time) ----------------
        def proj_qk(w_s, x_src, b_s, dst01, dst2, n0, nw):
            """Q^T/K^T for one t-chunk: out[j, t] = W.T[:, j].T @ x^T[:, t].
            The m=64 block (head 2) is computed twice via two col-tiled
            matmuls (array cols 0-63 / 64-127, concurrent) so dst2 holds
            the head-2 rows on BOTH partition halves.  PSUM->SBUF copies
            ride the DVE so the ACT engine stays free for the exp stream."""
            pt = ppool.tile([128, 512], F32, tag="ps", name="pt_qk")
            for ci in range(6):
                nc.tensor.matmul(
                    pt[0:128, 0:nw],
                    lhsT=w_s[:, ci, 0:128],
                    rhs=x_src[:, ci, n0:n0 + nw],
                    start=(ci == 0), stop=(ci == 5))
            nc.vector.tensor_scalar(
                out=dst01[:, n0:n0 + nw], in0=pt[0:128, 0:nw],
                scalar1=b_s[:, 0:1], scalar2=None, op0=mybir.AluOpType.add)
            pt2 = ppool.tile([128, 512], F32, tag="ps", name="pt_qk2")
            for half in range(2):
                for ci in range(6):
                    nc.tensor.matmul(
                        pt2[64 * half:64 * half + 64, 0:nw],
                        lhsT=w_s[:, ci, 128:192],
                        rhs=x_src[:, ci, n0:n0 + nw],
                        start=(ci == 0), stop=(ci == 5))
            nc.vector.tensor_scalar(
                out=dst2[:, n0:n0 + nw], in0=pt2[0:128, 0:nw],
                scalar1=b_s[:, 1:2], scalar2=None, op0=mybir.AluOpType.add)

        def proj_v(tt):
            pt = ppool.tile([128, 512], F32, tag="ps", name="pt_v")
            for ci in range(6):
                nc.tensor.matmul(
                    pt[:, 0:J],
                    lhsT=xkv_s[:, ci, tt * 128:(tt + 1) * 128],
                    rhs=wv_s[:, ci, :],
                    start=(ci == 0), stop=False)
            nc.tensor.matmul(
                pt[:, 0:J],
                lhsT=xkv1_s[0:1, tt * 128:(tt + 1) * 128],
                rhs=wv1_s[0:1, :],
                start=False, stop=True)
            for h in range(3):
                nc.vector.tensor_copy(v_s[h][:, tt, 0:64], pt[:, h * 64:(h + 1) * 64])

        spool = ctx.enter_context(tc.tile_pool(name="ystage", bufs=4))
        NQ = T // QCW

        def emit_normalize(qc):
            """yn = Y^T * broadcast(1/denom), reading the SBUF-staged Y'.

            The [1, 512] denominator row is spread to [64, 8] by DMA so the
            reciprocal runs on 64 DVE lanes, then DMA'd back to row form for
            the K=1 broadcast matmul.  The h0/h1 broadcasts are col-tiled
            into one concurrent slot.  The small DMAs ride the gpsimd queue."""
            q0 = qc * QCW
            ys3, rd3 = {}, {}
            for h in range(3):
                ys3[h] = stages.pop((qc, h))
                dsp = mpool.tile([64, 8], F32, tag="dsp", name="dsp")
                nc.gpsimd.dma_start(dsp[:, :], ys3[h][64:65, :])
                rsp = mpool.tile([64, 8], F16, tag="rsp", name="rsp")
                with nc.allow_low_precision(reason="1/denom consumed as fp16"):
                    nc.vector.reciprocal(rsp[:, :], dsp[:, :])
                rd = mpool.tile([65, QCW], F16, tag="rd", name="rd")
                nc.gpsimd.dma_start(rd[64:65, :], rsp[:, :])
                rd3[h] = rd
            bc01 = ppool.tile([128, 512], F32, tag="ps", name="bc01")
            for h in range(2):
                nc.tensor.matmul(
                    bc01[64 * h:64 * h + 64, 0:QCW],
                    lhsT=ones_s[64:65, 0:64],
                    rhs=rd3[h][64:65, 0:QCW],
                    start=True, stop=True)
            bc2 = ppool.tile([128, 512], F32, tag="ps", name="bc2")
            nc.tensor.matmul(
                bc2[0:64, 0:QCW],
                lhsT=ones_s[64:65, 0:64],
                rhs=rd3[2][64:65, 0:QCW],
                start=True, stop=True)
            for h in range(3):
                yn_ap = (yn01_s[64 * h:64 * h + 64, q0:q0 + QCW] if h < 2
                         else yn2_s[0:64, q0:q0 + QCW])
                bc_ap = bc01[64 * h:64 * h + 64, 0:QCW] if h < 2 else bc2[0:64, 0:QCW]
                nc.vector.tensor_tensor(
                    out=yn_ap, in0=ys3[h][0:64, :], in1=bc_ap,
                    op=mybir.AluOpType.mult)

        oq = itertools.cycle([nc.sync, nc.scalar, nc.gpsimd])

        def final_groups(qc, tail=False):
            """Output projection for q-chunk qc as 4 closures (one per
            128-row t-tile) so the attention loop can spread them out.
            In the tail, copies alternate DVE/ACT and the output DMA is
            striped over all three queues to drain fast."""
            q0 = qc * QCW

            def make(tt):
                def go():
                    o_sb = opool.tile([128, 768], F16, tag="osb", name="o_sb")
                    for oi, (n0, nw) in enumerate(_nchunks(768)):
                        op = ppool.tile([128, 512], F32, tag="ps", name="op")
                        nc.tensor.matmul(
                            op[0:128, 0:nw],
                            lhsT=yn01_s[:, tt * 128:(tt + 1) * 128],
                            rhs=wpT01_s[:, n0:n0 + nw],
                            start=True, stop=False)
                        nc.tensor.matmul(
                            op[0:128, 0:nw],
                            lhsT=yn2_s[:, tt * 128:(tt + 1) * 128],
                            rhs=wph_s[2][:, n0:n0 + nw],
                            start=False, stop=True)
                        if tail and (tt + oi) % 2:
                            nc.scalar.copy(o_sb[:, n0:n0 + nw], op[0:128, 0:nw])
                        else:
                            nc.vector.tensor_copy(o_sb[:, n0:n0 + nw], op[0:128, 0:nw])
                        eng = next(oq) if tail else nc.sync
                        eng.dma_start(
                            out[tt * 128:(tt + 1) * 128, n0:n0 + nw],
                            o_sb[:, n0:n0 + nw])
                return go
            return [make(tt) for tt in range(q0 // 128, (q0 + QCW) // 128)]

        def warmup(n):
            """n dummy full-array matmuls on a zero tile: fills the initial
            DMA-wait and trips the HAM clock gate to full rate (needs
            ~3.4us of sustained PE activity)."""
            wp_ps = ppool.tile([128, 512], F32, tag="ps", name="warm_ps")
            for i in range(n):
                nc.tensor.matmul(wp_ps[:, 0:512], lhsT=warm_s[:, 0:128],
                                 rhs=warm_s[:, 0:512], start=True, stop=True)

        # upfront: only what the first q-chunk needs immediately
        warmup(9)
        proj_qk(wk_s, xkv_s, bk_s, kt01_s, kt2_s, *kchunks[0])
        k_done = 1
        for tt in range(min(4, kk)):
            proj_v(tt)
        v_done = min(4, kk)
        proj_qk(wq_s, xt_s, bq_s, qt01_s, qt2_s, *qchunks[0])
        q_done = 1

        groups = _pairing(kk)
        stages = {}
        filler = []
        pend_pv = None

        def emit_pv():
            nonlocal pend_pv
            if pend_pv is None:
                return
            g, esd, yp = pend_pv
            pend_pv = None
            for h in range(3):
                for i, kkt in enumerate(g):
                    nc.tensor.matmul(
                        yp[h][0:65, 0:QCW],
                        lhsT=v_s[h][:, kkt, :],
                        rhs=esd[h][:, i * 512:(i + 1) * 512],
                        start=(kkt == 0), stop=(kkt == kk - 1))

        for qc in range(NQ):
            q0 = qc * QCW
            yps = {}
            for h in range(3):
                yps[h] = ppool.tile([128, 512], F32, tag="ps", name=f"yp{h}")
            for gi, g in enumerate(groups):
                kt0 = g[0]
                # just-in-time remaining projections (first q-chunk only)
                while k_done < len(kchunks) and kchunks[k_done][0] < (kt0 + 4) * 128:
                    proj_qk(wk_s, xkv_s, bk_s, kt01_s, kt2_s, *kchunks[k_done])
                    k_done += 1
                while v_done < kk and v_done <= kt0 + 3:
                    proj_v(v_done)
                    v_done += 1
                if gi == 3 and q_done <= qc + 1 and qc + 1 < NQ:
                    proj_qk(wq_s, xt_s, bq_s, qt01_s, qt2_s, *qchunks[qc + 1])
                    q_done = qc + 2
                # scores for heads 0/1 (row-packed, concurrent per key tile)
                sts = {}
                for h in range(2):
                    sts[h] = stpool.tile([128, 1024], F32, tag="stb", name=f"st{h}")
                for i, kkt in enumerate(g):
                    for h in range(2):
                        rows = slice(64 * h, 64 * h + 64)
                        nc.tensor.matmul(
                            sts[h][:, i * 512:(i + 1) * 512],
                            lhsT=kt01_s[rows, kkt * 128:(kkt + 1) * 128],
                            rhs=qt01_s[rows, q0:q0 + QCW],
                            start=True, stop=True)
                # previous group's PV rides here so the PE keeps streaming
                # while the ACT engine exps this group's scores
                emit_pv()
                # head-2 scores: the two key tiles of a pair use disjoint
                # partition halves of the duplicated K2/Q2 -> concurrent
                st2 = stpool.tile([128, 1024], F32, tag="stb", name="st2")
                sts[2] = st2
                for i, kkt in enumerate(g):
                    rows = slice(64 * i, 64 * i + 64)
                    nc.tensor.matmul(
                        st2[:, i * 512:(i + 1) * 512],
                        lhsT=kt2_s[rows, kkt * 128:(kkt + 1) * 128],
                        rhs=qt2_s[rows, q0:q0 + QCW],
                        start=True, stop=True)
                if gi == 1 and qc >= 1:
                    emit_normalize(qc - 1)
                elif filler:
                    filler.pop(0)()
                esd = {}
                for h in range(3):
                    es = espool.tile([128, 1024], F16, tag="es")
                    esd[h] = es
                    if g[-1] < clean_kk:
                        nc.scalar.activation(
                            out=es[:, 0:512 * len(g)], in_=sts[h][:, 0:512 * len(g)],
                            func=mybir.ActivationFunctionType.Exp,
                            bias=0.0, scale=SCALE)
                    else:
                        for i, kkt in enumerate(g):
                            nc.scalar.activation(
                                out=es[:, i * 512:(i + 1) * 512],
                                in_=sts[h][:, i * 512:(i + 1) * 512],
                                func=mybir.ActivationFunctionType.Exp,
                                bias=mb_s[:, kkt:kkt + 1], scale=SCALE)
                pend_pv = (g, esd, yps)
            emit_pv()
            # guards for small-kk shapes where the in-loop hooks never fired
            if qc >= 1 and (qc - 1, 0) in stages:
                emit_normalize(qc - 1)
            if q_done <= qc + 1 and qc + 1 < NQ:
                proj_qk(wq_s, xt_s, bq_s, qt01_s, qt2_s, *qchunks[qc + 1])
                q_done = qc + 2
            # stage Y' out of PSUM so the slots free for the next q-chunk
            for h in range(3):
                ys = spool.tile([65, QCW], F32, tag="ys", name=f"ys{h}")
                nc.vector.tensor_copy(ys[:, :], yps[h][0:65, 0:QCW])
                stages[(qc, h)] = ys
            if qc >= 1:
                filler.extend(final_groups(qc - 1))
        while filler:
            filler.pop(0)()
        emit_normalize(NQ - 1)
        for go in final_groups(NQ - 1, tail=True):
            go()

        if debug_taps:
            taps = [
                ("qt01", qt01_s[:, :], [128, T]),
                ("kt01", kt01_s[:, :], [128, tk]),
                ("v0", v_s[0][:, 0, :], [128, 65]),
                ("yn0", yn01_s[:, :], [128, T]),
                ("yn2", yn2_s[:, :], [65, T]),
            ]
            for nm, ap_t, shp in taps:
                dt_ = nc.dram_tensor(f"dbg_{nm}", shp, F16, kind="ExternalOutput").ap()
                nc.sync.dma_start(dt_, ap_t)

    nc.compile()
    return nc


def _prep_core_inputs(x, attn_mask1, Wq, bq, Wk, bk, Wv, bv, Wp, bp):
    """Host-side sharding: returns (in_maps, tk, share_x)."""
    x = np.asarray(x, np.float32)
    attn_mask1 = np.asarray(attn_mask1)
    Wq, Wk, Wv, Wp = (np.asarray(a, np.float32) for a in (Wq, Wk, Wv, Wp))
    bq, bk, bv, bp = (np.asarray(a, np.float32) for a in (bq, bk, bv, bp))

    ones = np.ones((1, T), np.float16)
    xts = [np.concatenate([x[b].T.astype(np.float16), ones], axis=0) for b in range(B)]

    if COMPACT:
        idxs = [np.nonzero(attn_mask1[b] != 0)[0] for b in range(B)]
        nmax = max(max(len(i) for i in idxs), 1)
        tk = ((nmax + 127) // 128) * 128
        share_x = False
        xkvs, mbs = [], []
        for b in range(B):
            idx = idxs[b]
            xg = np.zeros((tk, C), np.float16)
            xg[:len(idx)] = x[b][idx].astype(np.float16)
            row = np.ones((1, tk), np.float16)
            xkvs.append(np.concatenate([xg.T, row], axis=0))
            m = np.zeros(tk, np.float32)
            m[len(idx):] = MASK_NEG
            mbs.append(m)
    else:
        tk = T
        share_x = True
        xkvs = [None, None]
        mbs = [np.where(attn_mask1[b] != 0, 0.0, MASK_NEG).astype(np.float32)
               for b in range(B)]

    WqT, WkT, WvT, WpT = (W.T.astype(np.float16) for W in (Wq, Wk, Wv, Wp))

    in_maps = []
    for c in range(NCORES):
        b, g = c // GROUPS, c % GROUPS
        js = slice(g * J, (g + 1) * J)
        m = {
            "xt": xts[b],
            "wqT": np.ascontiguousarray(WqT[:, js]),
            "wkT": np.ascontiguousarray(WkT[:, js]),
            "wvT": np.concatenate([WvT[:, js], bv[js].astype(np.float16)[None, :]], axis=0),
            "bqv": np.ascontiguousarray(bq[js]),
            "bkv": np.ascontiguousarray(bk[js]),
            "mb": mbs[b],
            "wpT": np.ascontiguousarray(WpT[js, :]),
            "bp4": (bp / GROUPS).astype(np.float16),
        }
        if not share_x:
            m["xtkv"] = xkvs[b]
        in_maps.append(m)
    if COMPACT:
        clean_kk = min(len(i) for i in idxs) // 128
    else:
        clean_kk = 0
    return in_maps, tk, share_x, clean_kk


_CACHE = {}


def kernel(**inputs):
    in_maps, tk, share_x, clean_kk = _prep_core_inputs(**inputs)
    key = (tk, share_x, clean_kk)
    if key not in _CACHE:
        _CACHE[key] = build_nc(tk, share_x, clean_kk)
    nc = _CACHE[key]
    res = bass_utils.run_bass_kernel_spmd(nc, in_maps, list(range(NCORES)))
    out = np.zeros((B, T, C), np.float32)
    for c in range(NCORES):
        out[c // GROUPS] += res.results[c]["o"].astype(np.float32)
    return out


if __name__ == "__main__":
    rng = np.random.default_rng(0)
    ins = {
        "x": rng.standard_normal((B, T, C), dtype=np.float32),
        "attn_mask1": rng.integers(0, 2, size=(B, T)).astype(np.int32),
        "Wq": rng.standard_normal((C, C), dtype=np.float32) * 0.02,
        "bq": np.zeros(C, np.float32),
        "Wk": rng.standard_normal((C, C), dtype=np.float32) * 0.02,
        "bk": np.zeros(C, np.float32),
        "Wv": rng.standard_normal((C, C), dtype=np.float32) * 0.02,
        "bv": np.zeros(C, np.float32),
        "Wp": rng.standard_normal((C, C), dtype=np.float32) * 0.02,
        "bp": np.zeros(C, np.float32),
    }
    out = kernel(**ins)
    print(out.shape, out.dtype, np.abs(out).max())


# revision 9
# speedup vs baseline: 1.1311x; 1.1311x over previous
"""
Trainium2 Bass kernel for nn_CausalSelfAttention_5214090298017.

Reference computes (B=2, T=2048, C=768, H=12, HD=64):
    q,k,v = split_heads(x @ W{q,k,v}.T + b)          # [B,H,T,HD]
    att   = softmax(mask(q @ k.T / sqrt(HD)))        # key-padding mask from attn_mask1
    y     = (att @ v).merge_heads() @ Wp.T + bp      # [B,T,C]

Sharding: 8 cores = 2 (batch) x 4 (head-groups of 3 heads).  Each core
computes a partial output  sum_{h in group} (att_h @ v_h) @ Wp_rows_h
([T, C]); the host sums the 4 group partials per batch (row-parallel Wp)
and concatenates over batch.

Key layout trick: the host PERMUTES each batch's tokens so the unmasked
keys come first.  Q/K/V all project from the SAME [C, T] x^T tensor
(K/V read only the first tk columns), the key-padding mask reduces to
"key index >= n_valid", and the host un-permutes the output rows.  This
halves the HBM input traffic vs shipping a separate compacted K/V copy
-- the input load is DMA-bandwidth-bound (~200 GB/s/core), so bytes are
wall-clock.

Device-side details (per core):
  - Q^T/K^T stored [head_dim, T]; heads 0/1 packed on partitions 0-63 /
    64-127.  Per key tile, h0/h1 score matmuls write the two halves of
    ONE [128, 1024] PSUM tile: the tile scheduler keeps same-tile
    matmuls adjacent, and disjoint row-groups make them concurrent
    (1 slot for 2 heads).  Head 2's Q^T/K^T are stored twice (both
    partition halves, built free by col-tiled projection matmuls), so
    the h2 scores for a key-tile PAIR also run concurrently.
  - S^T tiles keep KEYS on partitions: the key-padding mask and the
    1/sqrt(HD) scale are applied for free by the Exp activation
    (per-partition bias + scale).
  - V stored [T_k, 65] per head with a ones-column: the PV matmul
    accumulates [Y^T | softmax-denominator] in one pass.  PV for group
    i is emitted between the h01 and h2 scores of group i+1 so the PE
    streams while the ACT engine exps.
  - Normalization (per-query 1/denom) spreads the denom row to 64 DVE
    lanes by DMA, reciprocals, and broadcasts back via K=1 matmuls
    (h0/h1 col-tiled into one slot).
  - Input DMA striped over the sync/ACT/gpsimd queues; tail output DMA
    striped the same way; dummy matmuls bridge the tail's normalize
    latency so the PE clock (HAM) stays at full rate for the final
    output projection.
Matmul operands are fp16 (full PE rate; fp32 PSUM accumulation).
"""

import itertools
import math
import os
import sys
from contextlib import ExitStack

import numpy as np

sys.path.insert(0, "/opt/trn_rl_repo")

import concourse.bass as bass  # noqa: E402,F401
import concourse.tile as tile  # noqa: E402
from concourse import bacc, mybir  # noqa: E402
from concourse import bass_utils  # noqa: E402

F32 = mybir.dt.float32
F16 = mybir.dt.float16
U16 = mybir.dt.uint16
ONE_F16 = 0x3C00  # 1.0 in fp16 bits (memset can't take fp16 directly)

B, T, C, H = 2, 2048, 768, 12
HD = C // H          # 64
GROUPS = 4           # head-groups (tensor parallel)
HPG = H // GROUPS    # 3 heads per group
J = HPG * HD         # 192 local channels
NCORES = 8
SCALE = 1.0 / math.sqrt(HD)
MASK_NEG = -30000.0  # exp(-30000 + small) == 0.0

COMPACT = os.environ.get("ATTN_NO_COMPACT", "") == ""
QCW = 512            # query chunk width for the attention phase


def _nchunks(n, cap=512, lo=256):
    """Split n (multiple of 128) into (start, width) chunks in [lo, cap]."""
    assert n % 128 == 0
    out, pos, rem = [], 0, n
    while rem > 0:
        w = min(cap, rem)
        if rem - w != 0 and rem - w < lo:
            w = max(lo, ((rem - lo) // 128) * 128)
        out.append((pos, w))
        pos += w
        rem -= w
    return out


def _pairing(kk):
    """Key tiles in pairs (last one single if kk is odd)."""
    gs = []
    i = 0
    while i + 1 < kk:
        gs.append((i, i + 1))
        i += 2
    if i < kk:
        gs.append((i,))
    return gs


def build_nc(tk, clean_kk=0, debug_taps=False):
    """Build the per-core Bass program.  tk = padded key count (mult of 128)."""
    kk = tk // 128

    nc = bacc.Bacc("TRN2", target_bir_lowering=False, debug=False)

    xt = nc.dram_tensor("xt", [769, T], F16, kind="ExternalInput").ap()
    wqT = nc.dram_tensor("wqT", [768, J], F16, kind="ExternalInput").ap()
    wkT = nc.dram_tensor("wkT", [768, J], F16, kind="ExternalInput").ap()
    wvT = nc.dram_tensor("wvT", [769, J], F16, kind="ExternalInput").ap()
    bqv = nc.dram_tensor("bqv", [J], F32, kind="ExternalInput").ap()
    bkv = nc.dram_tensor("bkv", [J], F32, kind="ExternalInput").ap()
    mb = nc.dram_tensor("mb", [tk], F32, kind="ExternalInput").ap()
    wpT = nc.dram_tensor("wpT", [J, 768], F16, kind="ExternalInput").ap()
    bp4 = nc.dram_tensor("bp4", [768], F16, kind="ExternalInput").ap()
    out = nc.dram_tensor("o", [T, 768], F16, kind="ExternalOutput").ap()

    with tile.TileContext(nc) as tc, ExitStack() as ctx:
        const = ctx.enter_context(tc.tile_pool(name="const", bufs=1))
        ppool = ctx.enter_context(tc.tile_pool(name="psum", bufs=4, space="PSUM"))
        stpool = ctx.enter_context(tc.tile_pool(name="stbig", bufs=2, space="PSUM"))
        espool = ctx.enter_context(tc.tile_pool(name="es", bufs=10))
        e2pool = ctx.enter_context(tc.tile_pool(name="es2", bufs=3))
        opool = ctx.enter_context(tc.tile_pool(name="osb", bufs=4))
        mpool = ctx.enter_context(tc.tile_pool(name="misc", bufs=3))

        # ---------------- persistent SBUF tensors ----------------
        xt_s = const.tile([128, 6, T], F16, tag="xt")
        xt1_s = const.tile([1, T], F16, tag="xt1")
        wq_s = const.tile([128, 6, J], F16, tag="wq")
        wk_s = const.tile([128, 6, J], F16, tag="wk")
        wv_s = const.tile([128, 6, J], F16, tag="wv")
        wv1_s = const.tile([1, J], F16, tag="wv1")
        bq_s = const.tile([128, 2], F32, tag="bq")
        bk_s = const.tile([128, 2], F32, tag="bk")
        mb_s = const.tile([128, kk], F32, tag="mb")
        wpT01_s = const.tile([128, 768], F16, tag="wp01")
        wph_s = {2: const.tile([65, 768], F16, tag="wp2", name="wp2")}
        ones_s = const.tile([65, 128], F16, tag="ones")
        # heads 0/1 packed on partitions 0-63 / 64-127; head 2 duplicated
        # on both partition halves (enables key-tile-pair concurrency)
        qt01_s = const.tile([128, T], F16, tag="qt01")
        qt2_s = const.tile([128, T], F16, tag="qt2")
        kt01_s = const.tile([128, tk], F16, tag="kt01")
        kt2_s = const.tile([128, tk], F16, tag="kt2")
        v_s = [const.tile([128, kk, 65], F16, tag=f"v{h}", name=f"v{h}") for h in range(3)]
        yn01_s = const.tile([128, T], F16, tag="yn01")
        yn2_s = const.tile([65, T], F16, tag="yn2")
        warm_s = const.tile([128, 512], F16, tag="warm")

        # ---------------- input DMAs striped over 3 queues ----------------
        dq = itertools.cycle([nc.sync, nc.scalar, nc.gpsimd])

        def dma(dst, src):
            next(dq).dma_start(dst, src)

        kchunks = _nchunks(tk)
        qchunks = _nchunks(T)
        for ci in range(6):
            dma(wk_s[:, ci, :], wkT[ci * 128:(ci + 1) * 128, :])
        n0, nw = kchunks[0]
        for ci in range(6):
            dma(xt_s[:, ci, n0:n0 + nw], xt[ci * 128:(ci + 1) * 128, n0:n0 + nw])
        for ci in range(6):
            dma(wv_s[:, ci, :], wvT[ci * 128:(ci + 1) * 128, :])
        dma(wv1_s[:, :], wvT[768:769, :])
        dma(bq_s[:, 0:1], bqv[0:128][:, None])
        dma(bq_s[0:64, 1:2], bqv[128:192][:, None])
        dma(bq_s[64:128, 1:2], bqv[128:192][:, None])
        dma(bk_s[:, 0:1], bkv[0:128][:, None])
        dma(bk_s[0:64, 1:2], bkv[128:192][:, None])
        dma(bk_s[64:128, 1:2], bkv[128:192][:, None])
        dma(mb_s[:, :], mb.rearrange("(o p) -> p o", p=128))
        dma(xt1_s[:, :], xt[768:769, :])
        for (n0, nw) in kchunks[1:]:
            for ci in range(6):
                dma(xt_s[:, ci, n0:n0 + nw], xt[ci * 128:(ci + 1) * 128, n0:n0 + nw])
        for ci in range(6):
            dma(wq_s[:, ci, :], wqT[ci * 128:(ci + 1) * 128, :])
        if tk < T:
            for ci in range(6):
                dma(xt_s[:, ci, tk:T], xt[ci * 128:(ci + 1) * 128, tk:T])
        dma(wpT01_s[:, :], wpT[0:128, :])
        dma(wph_s[2][0:64, :], wpT[128:192, :])
        dma(wph_s[2][64:65, :], bp4[None, :])

        nc.vector.memset(warm_s[:, :].bitcast(U16), 0)
        nc.vector.memset(ones_s[:, :].bitcast(U16), ONE_F16)
        nc.vector.memset(yn2_s[64:65, :].bitcast(U16), ONE_F16)
        for h in range(3):
            nc.vector.memset(v_s[h][:, :, 64:65].bitcast(U16), ONE_F16)
        # pre-load the ACT exp table while the input DMA streams (the
        # first real exp would otherwise pay the ~1.3us table load)
        wact = mpool.tile([64, 8], F16, tag="wact", name="wact")
        nc.scalar.activation(out=wact[:, :], in_=warm_s[0:64, 0:8],
                             func=mybir.ActivationFunctionType.Exp,
                             bias=0.0, scale=1.0)

        # ---------------- projections (emitted just-in-time) ----------------
        def proj_qk(w_s, b_s, dst01, dst2, n0, nw):
            """Q^T/K^T for one t-chunk: out[j, t] = W.T[:, j].T @ x^T[:, t].
            The m=64 block (head 2) is computed twice via two col-tiled
            matmuls (array cols 0-63 / 64-127, concurrent) so dst2 holds
            the head-2 rows on BOTH partition halves.  PSUM->SBUF copies
            ride the DVE so the ACT engine stays free for the exp stream."""
            pt = ppool.tile([128, 512], F32, tag="ps", name="pt_qk")
            for ci in range(6):
                nc.tensor.matmul(
                    pt[0:128, 0:nw],
                    lhsT=w_s[:, ci, 0:128],
                    rhs=xt_s[:, ci, n0:n0 + nw],
                    start=(ci == 0), stop=(ci == 5))
            nc.vector.tensor_scalar(
                out=dst01[:, n0:n0 + nw], in0=pt[0:128, 0:nw],
                scalar1=b_s[:, 0:1], scalar2=None, op0=mybir.AluOpType.add)
            pt2 = ppool.tile([128, 512], F32, tag="ps", name="pt_qk2")
            for half in range(2):
                for ci in range(6):
                    nc.tensor.matmul(
                        pt2[64 * half:64 * half + 64, 0:nw],
                        lhsT=w_s[:, ci, 128:192],
                        rhs=xt_s[:, ci, n0:n0 + nw],
                        start=(ci == 0), stop=(ci == 5))
            nc.vector.tensor_scalar(
                out=dst2[:, n0:n0 + nw], in0=pt2[0:128, 0:nw],
                scalar1=b_s[:, 1:2], scalar2=None, op0=mybir.AluOpType.add)

        def proj_v(tt):
            pt = ppool.tile([128, 512], F32, tag="ps", name="pt_v")
            for ci in range(6):
                nc.tensor.matmul(
                    pt[:, 0:J],
                    lhsT=xt_s[:, ci, tt * 128:(tt + 1) * 128],
                    rhs=wv_s[:, ci, :],
                    start=(ci == 0), stop=False)
            nc.tensor.matmul(
                pt[:, 0:J],
                lhsT=xt1_s[0:1, tt * 128:(tt + 1) * 128],
                rhs=wv1_s[0:1, :],
                start=False, stop=True)
            for h in range(3):
                nc.vector.tensor_copy(v_s[h][:, tt, 0:64], pt[:, h * 64:(h + 1) * 64])

        spool = ctx.enter_context(tc.tile_pool(name="ystage", bufs=4))
        NQ = T // QCW

        def emit_normalize(qc):
            """yn = Y^T * broadcast(1/denom), reading the SBUF-staged Y'.

            The [1, 512] denominator row is spread to [64, 8] by DMA so the
            reciprocal runs on 64 DVE lanes, then DMA'd back to row form for
            the K=1 broadcast matmul.  The h0/h1 broadcasts are col-tiled
            into one concurrent slot."""
            q0 = qc * QCW
            ys3, rd3 = {}, {}
            for h in range(3):
                ys3[h] = stages.pop((qc, h))
                dsp = mpool.tile([64, 8], F32, tag="dsp", name="dsp")
                nc.gpsimd.dma_start(dsp[:, :], ys3[h][64:65, :])
                rsp = mpool.tile([64, 8], F16, tag="rsp", name="rsp")
                with nc.allow_low_precision(reason="1/denom consumed as fp16"):
                    nc.vector.reciprocal(rsp[:, :], dsp[:, :])
                rd = mpool.tile([65, QCW], F16, tag="rd", name="rd")
                nc.gpsimd.dma_start(rd[64:65, :], rsp[:, :])
                rd3[h] = rd
            bc01 = ppool.tile([128, 512], F32, tag="ps", name="bc01")
            for h in range(2):
                nc.tensor.matmul(
                    bc01[64 * h:64 * h + 64, 0:QCW],
                    lhsT=ones_s[64:65, 0:64],
                    rhs=rd3[h][64:65, 0:QCW],
                    start=True, stop=True)
            bc2 = ppool.tile([128, 512], F32, tag="ps", name="bc2")
            nc.tensor.matmul(
                bc2[0:64, 0:QCW],
                lhsT=ones_s[64:65, 0:64],
                rhs=rd3[2][64:65, 0:QCW],
                start=True, stop=True)
            for h in range(3):
                yn_ap = (yn01_s[64 * h:64 * h + 64, q0:q0 + QCW] if h < 2
                         else yn2_s[0:64, q0:q0 + QCW])
                bc_ap = bc01[64 * h:64 * h + 64, 0:QCW] if h < 2 else bc2[0:64, 0:QCW]
                nc.vector.tensor_tensor(
                    out=yn_ap, in0=ys3[h][0:64, :], in1=bc_ap,
                    op=mybir.AluOpType.mult)

        oq = itertools.cycle([nc.sync, nc.scalar, nc.gpsimd])

        def final_groups(qc, tail=False):
            """Output projection for q-chunk qc as 4 closures (one per
            128-row t-tile) so the attention loop can spread them out.
            In the tail, copies alternate DVE/ACT and the output DMA is
            striped over all three queues to drain fast."""
            q0 = qc * QCW

            def make(tt):
                def go():
                    o_sb = opool.tile([128, 768], F16, tag="osb", name="o_sb")
                    for oi, (n0, nw) in enumerate(_nchunks(768)):
                        op = ppool.tile([128, 512], F32, tag="ps", name="op")
                        nc.tensor.matmul(
                            op[0:128, 0:nw],
                            lhsT=yn01_s[:, tt * 128:(tt + 1) * 128],
                            rhs=wpT01_s[:, n0:n0 + nw],
                            start=True, stop=False)
                        nc.tensor.matmul(
                            op[0:128, 0:nw],
                            lhsT=yn2_s[:, tt * 128:(tt + 1) * 128],
                            rhs=wph_s[2][:, n0:n0 + nw],
                            start=False, stop=True)
                        if tail and (tt + oi) % 2:
                            nc.scalar.copy(o_sb[:, n0:n0 + nw], op[0:128, 0:nw])
                        else:
                            nc.vector.tensor_copy(o_sb[:, n0:n0 + nw], op[0:128, 0:nw])
                        eng = next(oq) if tail else nc.sync
                        eng.dma_start(
                            out[tt * 128:(tt + 1) * 128, n0:n0 + nw],
                            o_sb[:, n0:n0 + nw])
                return go
            return [make(tt) for tt in range(q0 // 128, (q0 + QCW) // 128)]

        def warmup(n, read_back=False):
            """n dummy full-array matmuls on a zero tile: fills dependency
            gaps so the HAM clock gate stays at full rate.  read_back adds
            a tiny DVE read so the BIR verifier sees a consumer."""
            wp_ps = ppool.tile([128, 512], F32, tag="ps", name="warm_ps")
            for i in range(n):
                nc.tensor.matmul(wp_ps[:, 0:512], lhsT=warm_s[:, 0:128],
                                 rhs=warm_s[:, 0:512], start=True, stop=True)
            if read_back:
                junk = mpool.tile([1, 8], F16, tag="junk", name="junk")
                with nc.allow_low_precision(reason="dummy read"):
                    nc.vector.tensor_copy(junk[:, :], wp_ps[0:1, 0:8])

        # upfront: only what the first q-chunk needs immediately
        warmup(9)
        proj_qk(wk_s, bk_s, kt01_s, kt2_s, *kchunks[0])
        k_done = 1
        for tt in range(min(4, kk)):
            proj_v(tt)
        v_done = min(4, kk)
        proj_qk(wq_s, bq_s, qt01_s, qt2_s, *qchunks[0])
        q_done = 1

        groups = _pairing(kk)
        stages = {}
        filler = []
        pend_pv = None

        def emit_pv():
            nonlocal pend_pv
            if pend_pv is None:
                return
            g, esd, yp = pend_pv
            pend_pv = None
            for h in range(3):
                for i, kkt in enumerate(g):
                    nc.tensor.matmul(
                        yp[h][0:65, 0:QCW],
                        lhsT=v_s[h][:, kkt, :],
                        rhs=esd[(h, i)],
                        start=(kkt == 0), stop=(kkt == kk - 1))

        for qc in range(NQ):
            q0 = qc * QCW
            yps = {}
            for h in range(3):
                yps[h] = ppool.tile([128, 512], F32, tag="ps", name=f"yp{h}")
            for gi, g in enumerate(groups):
                kt0 = g[0]
                # just-in-time remaining projections (first q-chunk only)
                while k_done < len(kchunks) and kchunks[k_done][0] < (kt0 + 4) * 128:
                    proj_qk(wk_s, bk_s, kt01_s, kt2_s, *kchunks[k_done])
                    k_done += 1
                while v_done < kk and v_done <= kt0 + 3:
                    proj_v(v_done)
                    v_done += 1
                if gi == 3 and q_done <= qc + 1 and qc + 1 < NQ:
                    proj_qk(wq_s, bq_s, qt01_s, qt2_s, *qchunks[qc + 1])
                    q_done = qc + 2
                # scores: per key tile, heads 0/1 write the two halves of ONE
                # PSUM tile -> the scheduler keeps them adjacent and the
                # disjoint row-groups run them concurrently (1 slot / tile)
                sts = {}
                for i, kkt in enumerate(g):
                    st = stpool.tile([128, 1024], F32, tag="stb", name=f"st_{i}")
                    sts[i] = st
                    for h in range(2):
                        rows = slice(64 * h, 64 * h + 64)
                        nc.tensor.matmul(
                            st[:, 512 * h:512 * h + 512],
                            lhsT=kt01_s[rows, kkt * 128:(kkt + 1) * 128],
                            rhs=qt01_s[rows, q0:q0 + QCW],
                            start=True, stop=True)
                # previous group's PV rides here so the PE keeps streaming
                # while the ACT engine exps this group's scores
                emit_pv()
                # head-2 scores: the two key tiles of a pair use disjoint
                # partition halves of the duplicated K2/Q2 -> concurrent
                st2 = stpool.tile([128, 1024], F32, tag="stb", name="st2")
                for i, kkt in enumerate(g):
                    rows = slice(64 * i, 64 * i + 64)
                    nc.tensor.matmul(
                        st2[:, i * 512:(i + 1) * 512],
                        lhsT=kt2_s[rows, kkt * 128:(kkt + 1) * 128],
                        rhs=qt2_s[rows, q0:q0 + QCW],
                        start=True, stop=True)
                if gi == 1 and qc >= 1 and (qc - 1, 0) in stages:
                    emit_normalize(qc - 1)
                    filler.extend(final_groups(qc - 1))
                elif filler:
                    filler.pop(0)()
                esd = {}
                for i, kkt in enumerate(g):
                    for h in range(2):
                        es = espool.tile([128, 512], F16, tag="es")
                        esd[(h, i)] = es
                        nc.scalar.activation(
                            out=es[:, :], in_=sts[i][:, 512 * h:512 * h + 512],
                            func=mybir.ActivationFunctionType.Exp,
                            bias=mb_s[:, kkt:kkt + 1], scale=SCALE)
                es2 = e2pool.tile([128, 1024], F16, tag="es2")
                if g[-1] < clean_kk:
                    nc.scalar.activation(
                        out=es2[:, 0:512 * len(g)], in_=st2[:, 0:512 * len(g)],
                        func=mybir.ActivationFunctionType.Exp,
                        bias=0.0, scale=SCALE)
                else:
                    for i, kkt in enumerate(g):
                        nc.scalar.activation(
                            out=es2[:, i * 512:(i + 1) * 512],
                            in_=st2[:, i * 512:(i + 1) * 512],
                            func=mybir.ActivationFunctionType.Exp,
                            bias=mb_s[:, kkt:kkt + 1], scale=SCALE)
                for i in range(len(g)):
                    esd[(2, i)] = es2[:, i * 512:(i + 1) * 512]
                pend_pv = (g, esd, yps)
            emit_pv()
            # guard for small-kk shapes where the in-loop hook never fired
            if qc >= 1 and (qc - 1, 0) in stages:
                emit_normalize(qc - 1)
                filler.extend(final_groups(qc - 1))
            if q_done <= qc + 1 and qc + 1 < NQ:
                proj_qk(wq_s, bq_s, qt01_s, qt2_s, *qchunks[qc + 1])
                q_done = qc + 2
            # stage Y' out of PSUM so the slots free for the next q-chunk
            for h in range(3):
                ys = spool.tile([65, QCW], F32, tag="ys", name=f"ys{h}")
                nc.vector.tensor_copy(ys[:, :], yps[h][0:65, 0:QCW])
                stages[(qc, h)] = ys
        # ---------------- tail ----------------
        # dummy matmuls bridge the normalize latency (DMA+recip+DMA) so the
        # PE stays HAM-warm for the final output projections
        warmup(8, read_back=True)
        emit_normalize(NQ - 1)
        while filler:
            filler.pop(0)()
        for go in final_groups(NQ - 1, tail=True):
            go()

        if debug_taps:
            taps = [
                ("qt01", qt01_s[:, :], [128, T]),
                ("kt01", kt01_s[:, :], [128, tk]),
                ("v0", v_s[0][:, 0, :], [128, 65]),
                ("yn0", yn01_s[:, :], [128, T]),
                ("yn2", yn2_s[:, :], [65, T]),
            ]
            for nm, ap_t, shp in taps:
                dt_ = nc.dram_tensor(f"dbg_{nm}", shp, F16, kind="ExternalOutput").ap()
                nc.sync.dma_start(dt_, ap_t)

    nc.compile()
    return nc


def _prep_core_inputs(x, attn_mask1, Wq, bq, Wk, bk, Wv, bv, Wp, bp):
    """Host-side sharding: returns (in_maps, tk, clean_kk, perms)."""
    x = np.asarray(x, np.float32)
    attn_mask1 = np.asarray(attn_mask1)
    Wq, Wk, Wv, Wp = (np.asarray(a, np.float32) for a in (Wq, Wk, Wv, Wp))
    bq, bk, bv, bp = (np.asarray(a, np.float32) for a in (bq, bk, bv, bp))

    ones = np.ones((1, T), np.float16)
    if COMPACT:
        # permute tokens so unmasked keys come first; Q/K/V share one x^T
        idxs = [np.nonzero(attn_mask1[b] != 0)[0] for b in range(B)]
        perms = [np.concatenate([idxs[b], np.nonzero(attn_mask1[b] == 0)[0]])
                 for b in range(B)]
        nmax = max(max(len(i) for i in idxs), 1)
        tk = min(((nmax + 127) // 128) * 128, T)
        clean_kk = min(len(i) for i in idxs) // 128
        mbs = []
        for b in range(B):
            m = np.zeros(tk, np.float32)
            m[len(idxs[b]):] = MASK_NEG
            mbs.append(m)
    else:
        perms = [np.arange(T), np.arange(T)]
        tk = T
        clean_kk = 0
        mbs = [np.where(attn_mask1[b] != 0, 0.0, MASK_NEG).astype(np.float32)
               for b in range(B)]
    xts = [np.concatenate([x[b][perms[b]].T.astype(np.float16), ones], axis=0)
           for b in range(B)]

    WqT, WkT, WvT, WpT = (W.T.astype(np.float16) for W in (Wq, Wk, Wv, Wp))

    in_maps = []
    for c in range(NCORES):
        b, g = c // GROUPS, c % GROUPS
        js = slice(g * J, (g + 1) * J)
        m = {
            "xt": xts[b],
            "wqT": np.ascontiguousarray(WqT[:, js]),
            "wkT": np.ascontiguousarray(WkT[:, js]),
            "wvT": np.concatenate([WvT[:, js], bv[js].astype(np.float16)[None, :]], axis=0),
            "bqv": np.ascontiguousarray(bq[js]),
            "bkv": np.ascontiguousarray(bk[js]),
            "mb": mbs[b],
            "wpT": np.ascontiguousarray(WpT[js, :]),
            "bp4": (bp / GROUPS).astype(np.float16),
        }
        in_maps.append(m)
    return in_maps, tk, clean_kk, perms


_CACHE = {}


def kernel(**inputs):
    in_maps, tk, clean_kk, perms = _prep_core_inputs(**inputs)
    key = (tk, clean_kk)
    if key not in _CACHE:
        _CACHE[key] = build_nc(tk, clean_kk)
    nc = _CACHE[key]
    res = bass_utils.run_bass_kernel_spmd(nc, in_maps, list(range(NCORES)))
    out = np.zeros((B, T, C), np.float32)
    for c in range(NCORES):
        out[c // GROUPS][perms[c // GROUPS]] += res.results[c]["o"].astype(np.float32)
    return out


if __name__ == "__main__":
    rng = np.random.default_rng(0)
    ins = {
        "x": rng.standard_normal((B, T, C), dtype=np.float32),
        "attn_mask1": rng.integers(0, 2, size=(B, T)).astype(np.int32),
        "Wq": rng.standard_normal((C, C), dtype=np.float32) * 0.02,
        "bq": np.zeros(C, np.float32),
        "Wk": rng.standard_normal((C, C), dtype=np.float32) * 0.02,
        "bk": np.zeros(C, np.float32),
        "Wv": rng.standard_normal((C, C), dtype=np.float32) * 0.02,
        "bv": np.zeros(C, np.float32),
        "Wp": rng.standard_normal((C, C), dtype=np.float32) * 0.02,
        "bp": np.zeros(C, np.float32),
    }
    out = kernel(**ins)
    print(out.shape, out.dtype, np.abs(out).max())


# revision 14
# speedup vs baseline: 1.1960x; 1.0574x over previous
"""
Trainium2 Bass kernel for nn_CausalSelfAttention_5214090298017.

Reference computes (B=2, T=2048, C=768, H=12, HD=64):
    q,k,v = split_heads(x @ W{q,k,v}.T + b)          # [B,H,T,HD]
    att   = softmax(mask(q @ k.T / sqrt(HD)))        # key-padding mask from attn_mask1
    y     = (att @ v).merge_heads() @ Wp.T + bp      # [B,T,C]

Sharding: 8 cores = 2 (batch) x 4 (head-groups of 3 heads).  Each core
computes a partial output  sum_{h in group} (att_h @ v_h) @ Wp_rows_h
([T, C]); the host sums the 4 group partials per batch (row-parallel Wp)
and concatenates over batch.

Key layout trick: the host PERMUTES each batch's tokens so the unmasked
keys come first.  Q/K/V all project from the SAME [C, T] x^T tensor
(K/V read only the first tk columns), the key-padding mask reduces to
"key index >= n_valid", and the host un-permutes the output rows.  This
halves the HBM input traffic vs shipping a separate compacted K/V copy
-- the input load is DMA-bandwidth-bound (~200 GB/s/core), so bytes are
wall-clock.

Device-side details (per core):
  - Q^T/K^T stored [head_dim, T]; heads 0/1 packed on partitions 0-63 /
    64-127.  Per key tile, h0/h1 score matmuls write the two halves of
    ONE [128, 1024] PSUM tile: the tile scheduler keeps same-tile
    matmuls adjacent, and disjoint row-groups make them concurrent
    (1 slot for 2 heads).  Head 2's Q^T/K^T are stored twice (both
    partition halves, built free by col-tiled projection matmuls), so
    the h2 scores for a key-tile PAIR also run concurrently.
  - S^T tiles keep KEYS on partitions: the key-padding mask and the
    1/sqrt(HD) scale are applied for free by the Exp activation
    (per-partition bias + scale).
  - V stored [T_k, 65] per head with a ones-column: the PV matmul
    accumulates [Y^T | softmax-denominator] in one pass.  PV for group
    i is emitted between the h01 and h2 scores of group i+1 so the PE
    streams while the ACT engine exps.
  - Normalization (per-query 1/denom) spreads the denom row to 64 DVE
    lanes by DMA, reciprocals, and broadcasts back via K=1 matmuls
    (h0/h1 col-tiled into one slot).
  - Input DMA striped over the sync/ACT/gpsimd queues; tail output DMA
    striped the same way; dummy matmuls bridge the tail's normalize
    latency so the PE clock (HAM) stays at full rate for the final
    output projection.
Matmul operands are fp16 (full PE rate; fp32 PSUM accumulation).
"""

import itertools
import math
import os
import sys
from contextlib import ExitStack

import numpy as np

sys.path.insert(0, "/opt/trn_rl_repo")

import concourse.bass as bass  # noqa: E402,F401
import concourse.tile as tile  # noqa: E402
from concourse import bacc, mybir  # noqa: E402
from concourse import bass_utils  # noqa: E402

F32 = mybir.dt.float32
F16 = mybir.dt.float16
U16 = mybir.dt.uint16
ONE_F16 = 0x3C00  # 1.0 in fp16 bits (memset can't take fp16 directly)

B, T, C, H = 2, 2048, 768, 12
HD = C // H          # 64
GROUPS = 4           # head-groups (tensor parallel)
HPG = H // GROUPS    # 3 heads per group
J = HPG * HD         # 192 local channels
NCORES = 8
SCALE = 1.0 / math.sqrt(HD)
MASK_NEG = -30000.0  # exp(-30000 + small) == 0.0

COMPACT = os.environ.get("ATTN_NO_COMPACT", "") == ""
QCW = 512            # query chunk width for the attention phase


def _nchunks(n, cap=512, lo=256):
    """Split n (multiple of 128) into (start, width) chunks in [lo, cap]."""
    assert n % 128 == 0
    out, pos, rem = [], 0, n
    while rem > 0:
        w = min(cap, rem)
        if rem - w != 0 and rem - w < lo:
            w = max(lo, ((rem - lo) // 128) * 128)
        out.append((pos, w))
        pos += w
        rem -= w
    return out


def _pairing(kk):
    """Key tiles in pairs (last one single if kk is odd)."""
    gs = []
    i = 0
    while i + 1 < kk:
        gs.append((i, i + 1))
        i += 2
    if i < kk:
        gs.append((i,))
    return gs


def build_nc(tk, clean_kk=0, debug_taps=False):
    """Build the per-core Bass program.  tk = padded key count (mult of 128)."""
    kk = tk // 128

    nc = bacc.Bacc("TRN2", target_bir_lowering=False, debug=False)

    xt = nc.dram_tensor("xt", [769, T], F16, kind="ExternalInput").ap()
    wqT = nc.dram_tensor("wqT", [768, J], F16, kind="ExternalInput").ap()
    wkT = nc.dram_tensor("wkT", [768, J], F16, kind="ExternalInput").ap()
    wvT = nc.dram_tensor("wvT", [769, J], F16, kind="ExternalInput").ap()
    bqv = nc.dram_tensor("bqv", [J], F32, kind="ExternalInput").ap()
    bkv = nc.dram_tensor("bkv", [J], F32, kind="ExternalInput").ap()
    mb = nc.dram_tensor("mb", [tk], F32, kind="ExternalInput").ap()
    wpT = nc.dram_tensor("wpT", [J, 768], F16, kind="ExternalInput").ap()
    bp4 = nc.dram_tensor("bp4", [768], F16, kind="ExternalInput").ap()
    out = nc.dram_tensor("o", [T, 768], F16, kind="ExternalOutput").ap()

    with tile.TileContext(nc) as tc, ExitStack() as ctx:
        const = ctx.enter_context(tc.tile_pool(name="const", bufs=1))
        ppool = ctx.enter_context(tc.tile_pool(name="psum", bufs=4, space="PSUM"))
        stpool = ctx.enter_context(tc.tile_pool(name="stbig", bufs=2, space="PSUM"))
        espool = ctx.enter_context(tc.tile_pool(name="es", bufs=6))
        e2pool = ctx.enter_context(tc.tile_pool(name="es2", bufs=3))
        opool = ctx.enter_context(tc.tile_pool(name="osb", bufs=4))
        mpool = ctx.enter_context(tc.tile_pool(name="misc", bufs=3))

        # ---------------- persistent SBUF tensors ----------------
        xt_s = const.tile([128, 6, T], F16, tag="xt")
        xt1_s = const.tile([1, T], F16, tag="xt1")
        wq_s = const.tile([128, 6, J], F16, tag="wq")
        wk_s = const.tile([128, 6, J], F16, tag="wk")
        wv_s = const.tile([128, 6, J], F16, tag="wv")
        wv1_s = const.tile([1, J], F16, tag="wv1")
        bq_s = const.tile([128, 2], F32, tag="bq")
        bk_s = const.tile([128, 2], F32, tag="bk")
        mb_s = const.tile([128, kk], F32, tag="mb")
        wpT01_s = const.tile([128, 768], F16, tag="wp01")
        wph_s = {2: const.tile([65, 768], F16, tag="wp2", name="wp2")}
        ones_s = const.tile([65, 128], F16, tag="ones")
        # heads 0/1 packed on partitions 0-63 / 64-127; head 2 duplicated
        # on both partition halves (enables key-tile-pair concurrency)
        qt01_s = const.tile([128, T], F16, tag="qt01")
        qt2_s = const.tile([128, T], F16, tag="qt2")
        kt01_s = const.tile([128, tk], F16, tag="kt01")
        kt2_s = const.tile([128, tk], F16, tag="kt2")
        v_s = [const.tile([128, kk, 65], F16, tag=f"v{h}", name=f"v{h}") for h in range(3)]
        yn01_s = const.tile([128, T], F16, tag="yn01")
        yn2_s = const.tile([65, T], F16, tag="yn2")
        warm_s = const.tile([128, 512], F16, tag="warm")

        # ---------------- input DMAs striped over 3 queues ----------------
        dq = itertools.cycle([nc.sync, nc.scalar, nc.gpsimd])

        def dma(dst, src):
            next(dq).dma_start(dst, src)

        kchunks = _nchunks(tk)
        qchunks = _nchunks(T)
        for ci in range(6):
            dma(wk_s[:, ci, :], wkT[ci * 128:(ci + 1) * 128, :])
        n0, nw = kchunks[0]
        for ci in range(6):
            dma(xt_s[:, ci, n0:n0 + nw], xt[ci * 128:(ci + 1) * 128, n0:n0 + nw])
        for ci in range(6):
            dma(wv_s[:, ci, :], wvT[ci * 128:(ci + 1) * 128, :])
        dma(wv1_s[:, :], wvT[768:769, :])
        dma(bq_s[:, 0:1], bqv[0:128][:, None])
        dma(bq_s[0:64, 1:2], bqv[128:192][:, None])
        dma(bq_s[64:128, 1:2], bqv[128:192][:, None])
        dma(bk_s[:, 0:1], bkv[0:128][:, None])
        dma(bk_s[0:64, 1:2], bkv[128:192][:, None])
        dma(bk_s[64:128, 1:2], bkv[128:192][:, None])
        dma(mb_s[:, :], mb.rearrange("(o p) -> p o", p=128))
        dma(xt1_s[:, :], xt[768:769, :])
        for ci in range(6):
            dma(wq_s[:, ci, :], wqT[ci * 128:(ci + 1) * 128, :])
        for (n0, nw) in kchunks[1:]:
            for ci in range(6):
                dma(xt_s[:, ci, n0:n0 + nw], xt[ci * 128:(ci + 1) * 128, n0:n0 + nw])
        if tk < T:
            for ci in range(6):
                dma(xt_s[:, ci, tk:T], xt[ci * 128:(ci + 1) * 128, tk:T])
        dma(wpT01_s[:, :], wpT[0:128, :])
        dma(wph_s[2][0:64, :], wpT[128:192, :])
        dma(wph_s[2][64:65, :], bp4[None, :])

        nc.vector.memset(warm_s[:, :].bitcast(U16), 0)
        nc.vector.memset(ones_s[:, :].bitcast(U16), ONE_F16)
        nc.vector.memset(yn2_s[64:65, :].bitcast(U16), ONE_F16)
        for h in range(3):
            nc.vector.memset(v_s[h][:, :, 64:65].bitcast(U16), ONE_F16)
        # pre-load the ACT exp table while the input DMA streams (the
        # first real exp would otherwise pay the ~1.3us table load)
        wact = mpool.tile([64, 8], F16, tag="wact", name="wact")
        nc.scalar.activation(out=wact[:, :], in_=warm_s[0:64, 0:8],
                             func=mybir.ActivationFunctionType.Exp,
                             bias=0.0, scale=1.0)

        # ---------------- projections (emitted just-in-time) ----------------
        def proj_qk(w_s, b_s, dst01, dst2, n0, nw):
            """Q^T/K^T for one t-chunk: out[j, t] = W.T[:, j].T @ x^T[:, t].
            The m=64 block (head 2) is computed twice via two col-tiled
            matmuls (array cols 0-63 / 64-127, concurrent) so dst2 holds
            the head-2 rows on BOTH partition halves.  PSUM->SBUF copies
            ride the DVE so the ACT engine stays free for the exp stream."""
            pt = ppool.tile([128, 512], F32, tag="ps", name="pt_qk")
            for ci in range(6):
                nc.tensor.matmul(
                    pt[0:128, 0:nw],
                    lhsT=w_s[:, ci, 0:128],
                    rhs=xt_s[:, ci, n0:n0 + nw],
                    start=(ci == 0), stop=(ci == 5))
            nc.vector.tensor_scalar(
                out=dst01[:, n0:n0 + nw], in0=pt[0:128, 0:nw],
                scalar1=b_s[:, 0:1], scalar2=None, op0=mybir.AluOpType.add)
            pt2 = ppool.tile([128, 512], F32, tag="ps", name="pt_qk2")
            for half in range(2):
                for ci in range(6):
                    nc.tensor.matmul(
                        pt2[64 * half:64 * half + 64, 0:nw],
                        lhsT=w_s[:, ci, 128:192],
                        rhs=xt_s[:, ci, n0:n0 + nw],
                        start=(ci == 0), stop=(ci == 5))
            nc.vector.tensor_scalar(
                out=dst2[:, n0:n0 + nw], in0=pt2[0:128, 0:nw],
                scalar1=b_s[:, 1:2], scalar2=None, op0=mybir.AluOpType.add)

        def proj_v(tt):
            pt = ppool.tile([128, 512], F32, tag="ps", name="pt_v")
            for ci in range(6):
                nc.tensor.matmul(
                    pt[:, 0:J],
                    lhsT=xt_s[:, ci, tt * 128:(tt + 1) * 128],
                    rhs=wv_s[:, ci, :],
                    start=(ci == 0), stop=False)
            nc.tensor.matmul(
                pt[:, 0:J],
                lhsT=xt1_s[0:1, tt * 128:(tt + 1) * 128],
                rhs=wv1_s[0:1, :],
                start=False, stop=True)
            for h in range(3):
                nc.vector.tensor_copy(v_s[h][:, tt, 0:64], pt[:, h * 64:(h + 1) * 64])

        spool = ctx.enter_context(tc.tile_pool(name="ystage", bufs=4))
        NQ = T // QCW

        def emit_normalize(qc):
            """yn = Y^T * broadcast(1/denom), reading the SBUF-staged Y'.

            The [1, 512] denominator row is spread to [64, 8] by DMA so the
            reciprocal runs on 64 DVE lanes, then DMA'd back to row form for
            the K=1 broadcast matmul.  The h0/h1 broadcasts are col-tiled
            into one concurrent slot."""
            q0 = qc * QCW
            ys3, rd3 = {}, {}
            for h in range(3):
                ys3[h] = stages.pop((qc, h))
                dsp = mpool.tile([64, 8], F32, tag="dsp", name="dsp")
                nc.gpsimd.dma_start(dsp[:, :], ys3[h][64:65, :])
                rsp = mpool.tile([64, 8], F16, tag="rsp", name="rsp")
                with nc.allow_low_precision(reason="1/denom consumed as fp16"):
                    nc.vector.reciprocal(rsp[:, :], dsp[:, :])
                rd = mpool.tile([65, QCW], F16, tag="rd", name="rd")
                nc.gpsimd.dma_start(rd[64:65, :], rsp[:, :])
                rd3[h] = rd
            bc01 = ppool.tile([128, 512], F32, tag="ps", name="bc01")
            for h in range(2):
                nc.tensor.matmul(
                    bc01[64 * h:64 * h + 64, 0:QCW],
                    lhsT=ones_s[64:65, 0:64],
                    rhs=rd3[h][64:65, 0:QCW],
                    start=True, stop=True)
            bc2 = ppool.tile([128, 512], F32, tag="ps", name="bc2")
            nc.tensor.matmul(
                bc2[0:64, 0:QCW],
                lhsT=ones_s[64:65, 0:64],
                rhs=rd3[2][64:65, 0:QCW],
                start=True, stop=True)
            for h in range(3):
                yn_ap = (yn01_s[64 * h:64 * h + 64, q0:q0 + QCW] if h < 2
                         else yn2_s[0:64, q0:q0 + QCW])
                bc_ap = bc01[64 * h:64 * h + 64, 0:QCW] if h < 2 else bc2[0:64, 0:QCW]
                nc.vector.tensor_tensor(
                    out=yn_ap, in0=ys3[h][0:64, :], in1=bc_ap,
                    op=mybir.AluOpType.mult)

        oq = itertools.cycle([nc.sync, nc.scalar])

        def final_groups(qc, tail=False):
            """Output projection for q-chunk qc as 4 closures (one per
            128-row t-tile) so the attention loop can spread them out.
            PSUM->SBUF copies alternate DVE/ACT; in the tail the output
            DMA is striped over sync+ACT (gpsimd stays free for the
            normalize spread DMAs)."""
            q0 = qc * QCW

            def make(tt):
                def go():
                    o_sb = opool.tile([128, 768], F16, tag="osb", name="o_sb")
                    for oi, (n0, nw) in enumerate(_nchunks(768)):
                        op = ppool.tile([128, 512], F32, tag="ps", name="op")
                        nc.tensor.matmul(
                            op[0:128, 0:nw],
                            lhsT=yn01_s[:, tt * 128:(tt + 1) * 128],
                            rhs=wpT01_s[:, n0:n0 + nw],
                            start=True, stop=False)
                        nc.tensor.matmul(
                            op[0:128, 0:nw],
                            lhsT=yn2_s[:, tt * 128:(tt + 1) * 128],
                            rhs=wph_s[2][:, n0:n0 + nw],
                            start=False, stop=True)
                        if (tt + oi) % 2:
                            nc.scalar.copy(o_sb[:, n0:n0 + nw], op[0:128, 0:nw])
                        else:
                            nc.vector.tensor_copy(o_sb[:, n0:n0 + nw], op[0:128, 0:nw])
                        eng = next(oq) if tail else nc.sync
                        eng.dma_start(
                            out[tt * 128:(tt + 1) * 128, n0:n0 + nw],
                            o_sb[:, n0:n0 + nw])
                return go
            return [make(tt) for tt in range(q0 // 128, (q0 + QCW) // 128)]

        def warmup(n, read_back=False):
            """n dummy full-array matmuls on a zero tile: fills dependency
            gaps so the HAM clock gate stays at full rate.  read_back adds
            a tiny DVE read so the BIR verifier sees a consumer."""
            wp_ps = ppool.tile([128, 512], F32, tag="ps", name="warm_ps")
            for i in range(n):
                nc.tensor.matmul(wp_ps[:, 0:512], lhsT=warm_s[:, 0:128],
                                 rhs=warm_s[:, 0:512], start=True, stop=True)
            if read_back:
                junk = mpool.tile([1, 8], F16, tag="junk", name="junk")
                with nc.allow_low_precision(reason="dummy read"):
                    nc.vector.tensor_copy(junk[:, :], wp_ps[0:1, 0:8])

        # upfront: only what the first q-chunk needs immediately
        warmup(9)
        proj_qk(wk_s, bk_s, kt01_s, kt2_s, *kchunks[0])
        k_done = 1
        for tt in range(min(4, kk)):
            proj_v(tt)
        v_done = min(4, kk)
        proj_qk(wq_s, bq_s, qt01_s, qt2_s, *qchunks[0])
        q_done = 1

        groups = _pairing(kk)
        stages = {}
        filler = []
        pend_pv = None

        def emit_pv():
            nonlocal pend_pv
            if pend_pv is None:
                return
            g, esd, yp = pend_pv
            pend_pv = None
            for h in range(3):
                for i, kkt in enumerate(g):
                    nc.tensor.matmul(
                        yp[h][0:65, 0:QCW],
                        lhsT=v_s[h][:, kkt, :],
                        rhs=esd[(h, i)],
                        start=(kkt == 0), stop=(kkt == kk - 1))

        for qc in range(NQ):
            q0 = qc * QCW
            yps = {}
            for h in range(3):
                yps[h] = ppool.tile([128, 512], F32, tag="ps", name=f"yp{h}")
            for gi, g in enumerate(groups):
                kt0 = g[0]
                # just-in-time remaining projections (first q-chunk only)
                while k_done < len(kchunks) and kchunks[k_done][0] < (kt0 + 4) * 128:
                    proj_qk(wk_s, bk_s, kt01_s, kt2_s, *kchunks[k_done])
                    k_done += 1
                while v_done < kk and v_done <= kt0 + 3:
                    proj_v(v_done)
                    v_done += 1
                if gi == 3 and q_done <= qc + 1 and qc + 1 < NQ:
                    proj_qk(wq_s, bq_s, qt01_s, qt2_s, *qchunks[qc + 1])
                    q_done = qc + 2
                # scores: per key tile, heads 0/1 write the two halves of ONE
                # PSUM tile -> the scheduler keeps them adjacent and the
                # disjoint row-groups run them concurrently (1 slot / tile)
                sts = {}
                for i, kkt in enumerate(g):
                    st = stpool.tile([128, 1024], F32, tag="stb", name=f"st_{i}")
                    sts[i] = st
                    for h in range(2):
                        rows = slice(64 * h, 64 * h + 64)
                        nc.tensor.matmul(
                            st[:, 512 * h:512 * h + 512],
                            lhsT=kt01_s[rows, kkt * 128:(kkt + 1) * 128],
                            rhs=qt01_s[rows, q0:q0 + QCW],
                            start=True, stop=True)
                # previous group's PV rides here so the PE keeps streaming
                # while the ACT engine exps this group's scores
                emit_pv()
                # head-2 scores: the two key tiles of a pair use disjoint
                # partition halves of the duplicated K2/Q2 -> concurrent
                st2 = stpool.tile([128, 1024], F32, tag="stb", name="st2")
                for i, kkt in enumerate(g):
                    rows = slice(64 * i, 64 * i + 64)
                    nc.tensor.matmul(
                        st2[:, i * 512:(i + 1) * 512],
                        lhsT=kt2_s[rows, kkt * 128:(kkt + 1) * 128],
                        rhs=qt2_s[rows, q0:q0 + QCW],
                        start=True, stop=True)
                if gi == 1 and qc >= 1 and (qc - 1, 0) in stages:
                    emit_normalize(qc - 1)
                    filler.extend(final_groups(qc - 1))
                elif filler:
                    filler.pop(0)()
                esd = {}
                for i, kkt in enumerate(g):
                    # one wide exp covers both heads' halves: the mask bias
                    # depends only on the key tile, shared by h0/h1
                    es = espool.tile([128, 1024], F16, tag="es")
                    nc.scalar.activation(
                        out=es[:, :], in_=sts[i][:, :],
                        func=mybir.ActivationFunctionType.Exp,
                        bias=mb_s[:, kkt:kkt + 1], scale=SCALE)
                    for h in range(2):
                        esd[(h, i)] = es[:, 512 * h:512 * h + 512]
                es2 = e2pool.tile([128, 1024], F16, tag="es2")
                if g[-1] < clean_kk:
                    nc.scalar.activation(
                        out=es2[:, 0:512 * len(g)], in_=st2[:, 0:512 * len(g)],
                        func=mybir.ActivationFunctionType.Exp,
                        bias=0.0, scale=SCALE)
                else:
                    for i, kkt in enumerate(g):
                        nc.scalar.activation(
                            out=es2[:, i * 512:(i + 1) * 512],
                            in_=st2[:, i * 512:(i + 1) * 512],
                            func=mybir.ActivationFunctionType.Exp,
                            bias=mb_s[:, kkt:kkt + 1], scale=SCALE)
                for i in range(len(g)):
                    esd[(2, i)] = es2[:, i * 512:(i + 1) * 512]
                pend_pv = (g, esd, yps)
            emit_pv()
            # guard for small-kk shapes where the in-loop hook never fired
            if qc >= 1 and (qc - 1, 0) in stages:
                emit_normalize(qc - 1)
                filler.extend(final_groups(qc - 1))
            if q_done <= qc + 1 and qc + 1 < NQ:
                proj_qk(wq_s, bq_s, qt01_s, qt2_s, *qchunks[qc + 1])
                q_done = qc + 2
            # stage Y' out of PSUM so the slots free for the next q-chunk
            for h in range(3):
                ys = spool.tile([65, QCW], F32, tag="ys", name=f"ys{h}")
                nc.vector.tensor_copy(ys[:, :], yps[h][0:65, 0:QCW])
                stages[(qc, h)] = ys
        # ---------------- tail ----------------
        # dummy matmuls bridge the normalize latency (DMA+recip+DMA) so the
        # PE stays HAM-warm for the final output projections
        warmup(18, read_back=True)
        emit_normalize(NQ - 1)
        while filler:
            filler.pop(0)()
        for go in final_groups(NQ - 1, tail=True):
            go()

        if debug_taps:
            taps = [
                ("qt01", qt01_s[:, :], [128, T]),
                ("kt01", kt01_s[:, :], [128, tk]),
                ("v0", v_s[0][:, 0, :], [128, 65]),
                ("yn0", yn01_s[:, :], [128, T]),
                ("yn2", yn2_s[:, :], [65, T]),
            ]
            for nm, ap_t, shp in taps:
                dt_ = nc.dram_tensor(f"dbg_{nm}", shp, F16, kind="ExternalOutput").ap()
                nc.sync.dma_start(dt_, ap_t)

    nc.compile()
    return nc


def _prep_core_inputs(x, attn_mask1, Wq, bq, Wk, bk, Wv, bv, Wp, bp):
    """Host-side sharding: returns (in_maps, tk, clean_kk, perms)."""
    x = np.asarray(x, np.float32)
    attn_mask1 = np.asarray(attn_mask1)
    Wq, Wk, Wv, Wp = (np.asarray(a, np.float32) for a in (Wq, Wk, Wv, Wp))
    bq, bk, bv, bp = (np.asarray(a, np.float32) for a in (bq, bk, bv, bp))

    ones = np.ones((1, T), np.float16)
    if COMPACT:
        # permute tokens so unmasked keys come first; Q/K/V share one x^T
        idxs = [np.nonzero(attn_mask1[b] != 0)[0] for b in range(B)]
        perms = [np.concatenate([idxs[b], np.nonzero(attn_mask1[b] == 0)[0]])
                 for b in range(B)]
        nmax = max(max(len(i) for i in idxs), 1)
        tk = min(((nmax + 127) // 128) * 128, T)
        clean_kk = min(len(i) for i in idxs) // 128
        mbs = []
        for b in range(B):
            m = np.zeros(tk, np.float32)
            m[len(idxs[b]):] = MASK_NEG
            mbs.append(m)
    else:
        perms = [np.arange(T), np.arange(T)]
        tk = T
        clean_kk = 0
        mbs = [np.where(attn_mask1[b] != 0, 0.0, MASK_NEG).astype(np.float32)
               for b in range(B)]
    xts = [np.concatenate([x[b][perms[b]].T.astype(np.float16), ones], axis=0)
           for b in range(B)]

    WqT, WkT, WvT, WpT = (W.T.astype(np.float16) for W in (Wq, Wk, Wv, Wp))

    in_maps = []
    for c in range(NCORES):
        b, g = c // GROUPS, c % GROUPS
        js = slice(g * J, (g + 1) * J)
        m = {
            "xt": xts[b],
            "wqT": np.ascontiguousarray(WqT[:, js]),
            "wkT": np.ascontiguousarray(WkT[:, js]),
            "wvT": np.concatenate([WvT[:, js], bv[js].astype(np.float16)[None, :]], axis=0),
            "bqv": np.ascontiguousarray(bq[js]),
            "bkv": np.ascontiguousarray(bk[js]),
            "mb": mbs[b],
            "wpT": np.ascontiguousarray(WpT[js, :]),
            "bp4": (bp / GROUPS).astype(np.float16),
        }
        in_maps.append(m)
    return in_maps, tk, clean_kk, perms


_CACHE = {}


def kernel(**inputs):
    in_maps, tk, clean_kk, perms = _prep_core_inputs(**inputs)
    key = (tk, clean_kk)
    if key not in _CACHE:
        _CACHE[key] = build_nc(tk, clean_kk)
    nc = _CACHE[key]
    res = bass_utils.run_bass_kernel_spmd(nc, in_maps, list(range(NCORES)))
    out = np.zeros((B, T, C), np.float32)
    for c in range(NCORES):
        out[c // GROUPS][perms[c // GROUPS]] += res.results[c]["o"].astype(np.float32)
    return out


if __name__ == "__main__":
    rng = np.random.default_rng(0)
    ins = {
        "x": rng.standard_normal((B, T, C), dtype=np.float32),
        "attn_mask1": rng.integers(0, 2, size=(B, T)).astype(np.int32),
        "Wq": rng.standard_normal((C, C), dtype=np.float32) * 0.02,
        "bq": np.zeros(C, np.float32),
        "Wk": rng.standard_normal((C, C), dtype=np.float32) * 0.02,
        "bk": np.zeros(C, np.float32),
        "Wv": rng.standard_normal((C, C), dtype=np.float32) * 0.02,
        "bv": np.zeros(C, np.float32),
        "Wp": rng.standard_normal((C, C), dtype=np.float32) * 0.02,
        "bp": np.zeros(C, np.float32),
    }
    out = kernel(**ins)
    print(out.shape, out.dtype, np.abs(out).max())
